# revision 31
# baseline (speedup 1.0000x reference)
"""Multi-head attention (B=2, S=2048, D=1024, H=16) on 8 trn2 NeuronCores.

Sharding: core c = (batch b=c//4, head-group hg=c%4, heads hg*4..hg*4+3).
 - data-parallel over batch, tensor-parallel over heads (Megatron-style):
   wq/wk/wv column-sharded, wo row-sharded; partial out projections summed
   host-side at unshard time; bo added host-side.
Device kernel (per core, all matmuls in fp32r = TF32, fp32 accumulate):
 - q_T, k_T computed in transposed layout [head*64, token] so the QKV biases
   are per-partition; v computed in natural layout [token, head*64] augmented
   with a ones column per head (so the attention matmul's 65th output row is
   the softmax denominator Z for free).
 - scores s_T[k, q] per (head, q-chunk) on PE; exp on ACT straight out of
   PSUM (scale=1/8 folded); attn_T accumulated over k-tiles on PE.
 - normalization: recip(Z) on DVE, partition-broadcast via a K=1 PE outer
   product, applied by DVE. Weights written to DRAM as w_T[h, k, q] (2KB
   contiguous bursts); host transposes at unshard time.
"""
import sys
import os

sys.path.insert(0, "/opt/trn_rl_repo")

import numpy as np

B = 2
S = 2048
D = 1024
H = 16
DH = 64          # head dim
HPC = 4          # heads per core
NCORES = 8
P = 128          # partitions
QC = 512         # q-chunk (tokens per scores/attn matmul rhs)
NQC = S // QC    # 4
NKT = S // P     # 16 k-tiles
NDT = D // P     # 8 contraction tiles for projections

_cached = {}


def _build_nc():
    import concourse.tile as tile
    import concourse.mybir as mybir
    from concourse import bacc
    from concourse.bass import ds, ts

    f32 = mybir.dt.float32
    f32r = mybir.dt.float32r
    AF = mybir.ActivationFunctionType

    nc = bacc.Bacc("TRN2", target_bir_lowering=False, debug=False,
                   num_devices=NCORES)

    # ---- DRAM I/O ----
    xT = nc.dram_tensor("xT", [D, S], f32r, kind="ExternalInput").ap()
    kxT = nc.dram_tensor("kxT", [D, S], f32r, kind="ExternalInput").ap()
    vxT = nc.dram_tensor("vxT", [D, S], f32r, kind="ExternalInput").ap()
    wq_s = nc.dram_tensor("wq_s", [D, HPC * DH], f32r, kind="ExternalInput").ap()
    wk_s = nc.dram_tensor("wk_s", [D, HPC * DH], f32r, kind="ExternalInput").ap()
    wv_s = nc.dram_tensor("wv_s", [D, HPC * DH], f32r, kind="ExternalInput").ap()
    wo_s = nc.dram_tensor("wo_s", [HPC * DH, D], f32r, kind="ExternalInput").ap()
    bq_s = nc.dram_tensor("bq_s", [HPC * DH, 1], f32, kind="ExternalInput").ap()
    bk_s = nc.dram_tensor("bk_s", [HPC * DH, 1], f32, kind="ExternalInput").ap()
    bv_s = nc.dram_tensor("bv_s", [1, HPC * DH], f32, kind="ExternalInput").ap()

    w_out = nc.dram_tensor("w_out", [HPC, S, S], f32, kind="ExternalOutput").ap()
    out_p = nc.dram_tensor("out_p", [S, D], f32, kind="ExternalOutput").ap()

    with tile.TileContext(nc) as tc, \
         nc.allow_low_precision(reason="fp32r (tf32) matmul inputs are intentional"):
        import contextlib
        stack = contextlib.ExitStack()
        with stack:
            singles = stack.enter_context(tc.tile_pool(name="singles", bufs=1))
            wpool = stack.enter_context(tc.tile_pool(name="wpool", bufs=1))
            qkv = stack.enter_context(tc.tile_pool(name="qkv", bufs=1))
            # phase-A-only pools, released before attention pools allocate
            stack_a = stack.enter_context(contextlib.ExitStack())
            wproj = stack_a.enter_context(tc.tile_pool(name="wproj", bufs=1))
            xin = stack_a.enter_context(tc.tile_pool(name="xin", bufs=8))
            ps_proj = stack_a.enter_context(
                tc.tile_pool(name="ps_proj", bufs=2, space="PSUM"))

            # ---- weight slices & biases into SBUF ----
            wq_sb = wproj.tile([P, NDT * 256], f32r)   # [128, (kt 8, col 256)]
            wk_sb = wproj.tile([P, NDT * 256], f32r)
            wv_sb = wproj.tile([P, NDT * 256], f32r)
            wo_sb = wpool.tile([P, 2 * D], f32r)       # [128, (pr 2, 1024)]
            nc.sync.dma_start(wq_sb[:].rearrange("p (kt c) -> p kt c", c=256),
                              wq_s.rearrange("(kt p) c -> p kt c", p=P))
            nc.sync.dma_start(wk_sb[:].rearrange("p (kt c) -> p kt c", c=256),
                              wk_s.rearrange("(kt p) c -> p kt c", p=P))
            nc.sync.dma_start(wv_sb[:].rearrange("p (kt c) -> p kt c", c=256),
                              wv_s.rearrange("(kt p) c -> p kt c", p=P))
            nc.sync.dma_start(wo_sb[:].rearrange("p (pr c) -> p pr c", c=D),
                              wo_s.rearrange("(pr p) c -> p pr c", p=P))
            bq_sb = singles.tile([P, 2], f32)
            bk_sb = singles.tile([P, 2], f32)
            nc.sync.dma_start(bq_sb[:].rearrange("p (pr o) -> p pr o", o=1),
                              bq_s.rearrange("(pr p) o -> p pr o", p=P))
            nc.sync.dma_start(bk_sb[:].rearrange("p (pr o) -> p pr o", o=1),
                              bk_s.rearrange("(pr p) o -> p pr o", p=P))
            bv_bc = singles.tile([P, HPC * DH], f32)
            nc.sync.dma_start(bv_bc[:], bv_s[0, :].partition_broadcast(P))

            ones_col_f = singles.tile([P, 1], f32)     # fp32 ones source
            nc.vector.memset(ones_col_f[:], 1.0)
            ones_row_f = singles.tile([1, P], f32)
            nc.vector.memset(ones_row_f[:], 1.0)
            ones_row = singles.tile([1, P], f32r)      # lhsT for PE broadcast
            nc.vector.tensor_copy(ones_row[:], ones_row_f[:])

            # persistent activations
            q_T = [qkv.tile([P, S], f32r, name=f"q_T{i}") for i in range(2)]
            k_T = [qkv.tile([P, S], f32r, name=f"k_T{i}") for i in range(2)]
            v_aug = [qkv.tile([P, HPC * (DH + 1)], f32r, name=f"v_aug{i}")
                     for i in range(NKT)]
            attn_all = [qkv.tile([P, S], f32r, name=f"attn_all{i}") for i in range(2)]

            # ---- phase A: projections ----
            aparts = os.environ.get("KERNEL_A_PARTS", "all")

            def proj_T(dst, w_sb, b_sb, src_dram):
                tiles = []
                for kt in range(NDT):
                    t = xin.tile([P, S], f32r, tag="xin")
                    nc.sync.dma_start(t[:], src_dram[ds(kt * P, P), :])
                    tiles.append(t)
                if aparts == "x":
                    return
                for pr in range(2):
                    for tc4 in range(NQC):
                        pm = ps_proj.tile([P, QC], f32, tag="pp")
                        for kt in range(NDT):
                            nc.tensor.matmul(
                                pm[:],
                                w_sb[:, ds(kt * 256 + pr * P, P)],
                                tiles[kt][:, ds(tc4 * QC, QC)],
                                start=(kt == 0), stop=(kt == NDT - 1))
                        if aparts == "xm":
                            continue
                        nc.vector.tensor_scalar_add(dst[pr][:, ds(tc4 * QC, QC)],
                                                    pm[:], b_sb[:, ds(pr, 1)])

            if aparts != "w":
                proj_T(k_T, wk_sb, bk_sb, kxT)
                proj_T(q_T, wq_sb, bq_sb, xT)

            # v in natural layout, augmented ones column per head
            skip_v = aparts in ("w", "x", "xm")
            vtiles = []
            for kt in (range(NDT) if not skip_v else []):
                t = xin.tile([P, S], f32r, tag="xin")
                nc.sync.dma_start(t[:], vxT[ds(kt * P, P), :])
                vtiles.append(t)
            for tk in (range(NKT) if not skip_v else []):
                pm = ps_proj.tile([P, HPC * DH], f32, tag="pp")
                for kt in range(NDT):
                    nc.tensor.matmul(
                        pm[:],
                        vtiles[kt][:, ds(tk * P, P)],
                        wv_sb[:, ds(kt * 256, 256)],
                        start=(kt == 0), stop=(kt == NDT - 1))
                va = v_aug[tk]
                va3 = va[:].rearrange("p (h c) -> p h c", c=DH + 1)
                nc.vector.tensor_add(
                    va3[:, :, 0:DH],
                    pm[:].rearrange("p (h c) -> p h c", c=DH),
                    bv_bc[:].rearrange("p (h c) -> p h c", c=DH))
                nc.vector.tensor_copy(
                    va3[:, :, DH:DH + 1],
                    ones_col_f[:, None, 0:1].broadcast_to([P, HPC, 1]))

            # ---- release phase-A pools, allocate attention pools ----
            stack_a.close()
            ps_scores = stack.enter_context(
                tc.tile_pool(name="ps_scores", bufs=2, space="PSUM"))
            ps_attn = stack.enter_context(
                tc.tile_pool(name="ps_attn", bufs=2, space="PSUM"))
            ps_bc = stack.enter_context(
                tc.tile_pool(name="ps_bc", bufs=2, space="PSUM"))
            ps_out = stack.enter_context(
                tc.tile_pool(name="ps_out", bufs=2, space="PSUM"))
            expp = stack.enter_context(tc.tile_pool(name="expp", bufs=6))
            woutp = stack.enter_context(tc.tile_pool(name="woutp", bufs=4))
            small = stack.enter_context(tc.tile_pool(name="small", bufs=4))
            outp = stack.enter_context(tc.tile_pool(name="outp", bufs=3))

            # ---- phase B: attention per (head, q-chunk) ----
            phases = os.environ.get("KERNEL_PHASES", "abc")
            if skip_v:
                phases = ""
            for h in (range(HPC) if "b" in phases else []):
                pr = h // 2
                off = (h % 2) * DH
                for qc in range(NQC):
                    ecs = []
                    for ktg in range(4):
                        ec = expp.tile([P, 4 * QC], f32r, tag="ec")
                        for j in range(4):
                            kt = ktg * 4 + j
                            pss = ps_scores.tile([P, QC], f32, tag="ps")
                            nc.tensor.matmul(
                                pss[:],
                                k_T[pr][off:off + DH, ds(kt * P, P)],
                                q_T[pr][off:off + DH, ds(qc * QC, QC)],
                                start=True, stop=True)
                            nc.scalar.activation(ec[:, ds(j * QC, QC)], pss[:],
                                                 AF.Exp, scale=0.125)
                        ecs.append(ec)
                    bparts = os.environ.get("KERNEL_B_PARTS", "all")
                    if bparts == "se":
                        continue
                    # attn_T (+Z row) accumulated over all 16 k-tiles
                    pa = ps_attn.tile([DH + 1, QC], f32, tag="pa")
                    for ktg in range(4):
                        for j in range(4):
                            kt = ktg * 4 + j
                            nc.tensor.matmul(
                                pa[:],
                                v_aug[kt][:, ds(h * (DH + 1), DH + 1)],
                                ecs[ktg][:, ds(j * QC, QC)],
                                start=(kt == 0), stop=(kt == NKT - 1))
                    # recip(Z) and partition-broadcast via K=1 outer product
                    tz = small.tile([1, QC], f32r, tag="tz")
                    nc.vector.reciprocal(tz[:], pa[DH:DH + 1, :])
                    pb = ps_bc.tile([P, QC], f32, tag="pb")
                    nc.tensor.matmul(pb[:], ones_row[:], tz[:], start=True, stop=True)
                    # SBUF copy of the broadcast factor (GpSimd can't read PSUM)
                    pb_s = small.tile([P, QC], f32, tag="pb_s")
                    nc.vector.tensor_copy(pb_s[:], pb[:])
                    # normalize attn rows into attn_all
                    nc.vector.tensor_mul(
                        attn_all[pr][off:off + DH, ds(qc * QC, QC)],
                        pa[0:DH, :], pb_s[0:DH, :])
                    if bparts == "sea":
                        continue
                    # normalize weights into fp32 staging tiles, DMA out
                    # (split DVE / GpSimd to balance engine load)
                    for ktg in range(4):
                        ec = ecs[ktg]
                        ecv = ec[:].rearrange("p (j q) -> p j q", j=4)
                        wn = woutp.tile([P, 4 * QC], f32, tag="wn")
                        wnv = wn[:].rearrange("p (j q) -> p j q", j=4)
                        if ktg % 2 == 0:
                            nc.vector.tensor_mul(
                                wnv, ecv,
                                pb[:, None, :].broadcast_to([P, 4, QC]))
                        else:
                            nc.gpsimd.tensor_mul(
                                wnv, ecv,
                                pb_s[:, None, :].broadcast_to([P, 4, QC]))
                        nc.sync.dma_start(
                            w_out[h, ds(ktg * 4 * P, 4 * P), ds(qc * QC, QC)]
                            .rearrange("(j p) q -> p j q", p=P),
                            wnv)

            # ---- phase C: output projection (partial) ----
            for tk in (range(NKT) if "c" in phases else []):
                for oc in range(2):
                    po = ps_out.tile([P, QC], f32, tag="po")
                    for pr in range(2):
                        nc.tensor.matmul(
                            po[:],
                            attn_all[pr][:, ds(tk * P, P)],
                            wo_sb[:, ds(pr * D + oc * QC, QC)],
                            start=(pr == 0), stop=(pr == 1))
                    to = outp.tile([P, QC], f32, tag="to")
                    nc.vector.tensor_copy(to[:], po[:])
                    nc.sync.dma_start(out_p[tk * P:(tk + 1) * P, ds(oc * QC, QC)], to[:])

    nc.compile()
    return nc


def _get_nc():
    if "nc" not in _cached:
        _cached["nc"] = _build_nc()
    return _cached["nc"]


def _get_sharded():
    """Build (once) a jitted shard_map callable over the bass module,
    mirroring concourse.bass2jax.run_bass_via_pjrt but cached so repeat
    calls skip retracing, and usable for device-time benchmarking."""
    if "sharded" in _cached:
        return _cached["sharded"]
    import jax
    import jax.numpy as jnp
    from jax.sharding import Mesh, PartitionSpec
    from jax.experimental.shard_map import shard_map
    import concourse.mybir as mybir
    from concourse.bass2jax import (_bass_exec_p, install_neuronx_cc_hook,
                                    partition_id_tensor)

    nc = _get_nc()
    install_neuronx_cc_hook()

    partition_name = (nc.partition_id_tensor.name
                      if nc.partition_id_tensor else None)
    in_names, out_names, out_avals, zero_shapes = [], [], [], []
    for alloc in nc.m.functions[0].allocations:
        if not isinstance(alloc, mybir.MemoryLocationSet):
            continue
        name = alloc.memorylocations[0].name
        if alloc.kind == "ExternalInput":
            if name == partition_name:
                continue
            in_names.append(name)
        elif alloc.kind == "ExternalOutput":
            shape = tuple(alloc.tensor_shape)
            dtype = mybir.dt.np(alloc.dtype)
            out_names.append(name)
            out_avals.append(jax.core.ShapedArray(shape, dtype))
            zero_shapes.append((shape, dtype))
    n_params = len(in_names)
    all_names = in_names + out_names
    if partition_name is not None:
        all_names = all_names + [partition_name]
    donate = tuple(range(n_params, n_params + len(out_names)))

    def _body(*args):
        operands = list(args)
        if partition_name is not None:
            operands.append(partition_id_tensor())
        outs = _bass_exec_p.bind(
            *operands,
            out_avals=tuple(out_avals),
            in_names=tuple(all_names),
            out_names=tuple(out_names),
            lowering_input_output_aliases=(),
            sim_require_finite=True,
            sim_require_nnan=True,
            nc=nc,
        )
        return tuple(outs)

    devices = jax.devices()[:NCORES]
    mesh = Mesh(np.asarray(devices), ("core",))
    nio = n_params + len(out_names)
    sharded = jax.jit(
        shard_map(_body, mesh=mesh,
                  in_specs=(PartitionSpec("core"),) * nio,
                  out_specs=(PartitionSpec("core"),) * len(out_names),
                  check_rep=False),
        donate_argnums=donate, keep_unused=True)

    out_sh = jax.sharding.NamedSharding(mesh, PartitionSpec("core"))

    zero_fns = [
        jax.jit(lambda s=shape, d=dtype: jnp.zeros((NCORES * s[0], *s[1:]), d),
                out_shardings=out_sh)
        for shape, dtype in zero_shapes
    ]

    def make_zeros():
        return [f() for f in zero_fns]

    _cached["sharded"] = dict(
        fn=sharded, in_names=in_names, out_names=out_names,
        out_avals=out_avals, make_zeros=make_zeros, mesh=mesh, sharding=out_sh)
    return _cached["sharded"]


def _run_sharded(in_maps):
    import jax
    sh = _get_sharded()
    concat_in = [
        np.concatenate([np.asarray(in_maps[c][name]) for c in range(NCORES)], axis=0)
        for name in sh["in_names"]
    ]
    dev_in = [jax.device_put(a, sh["sharding"]) for a in concat_in]
    zeros = sh["make_zeros"]()
    out_arrs = sh["fn"](*dev_in, *zeros)
    _cached["last_dev_in"] = dev_in
    gathered = [
        np.asarray(a).reshape(NCORES, *sh["out_avals"][i].shape)
        for i, a in enumerate(out_arrs)
    ]
    return [
        {name: gathered[i][c] for i, name in enumerate(sh["out_names"])}
        for c in range(NCORES)
    ]


def bench(n_iters=5):
    """Re-execute the compiled NEFF with device-resident inputs; returns list
    of per-iteration wall seconds (dispatch + HW exec, no host transfers)."""
    import time
    import jax
    sh = _get_sharded()
    dev_in = _cached["last_dev_in"]
    times = []
    for _ in range(n_iters):
        zeros = sh["make_zeros"]()
        for z in zeros:
            z.block_until_ready()
        t0 = time.perf_counter()
        outs = sh["fn"](*dev_in, *zeros)
        for o in outs:
            o.block_until_ready()
        times.append(time.perf_counter() - t0)
    return times


def kernel(queries, keys, values, wq, bq, wk, bk, wv, bv, wo, bo):
    queries = np.ascontiguousarray(np.asarray(queries, dtype=np.float32))
    keys = np.ascontiguousarray(np.asarray(keys, dtype=np.float32))
    values = np.ascontiguousarray(np.asarray(values, dtype=np.float32))
    wq = np.asarray(wq, dtype=np.float32)
    wk = np.asarray(wk, dtype=np.float32)
    wv = np.asarray(wv, dtype=np.float32)
    wo = np.asarray(wo, dtype=np.float32)
    bq = np.asarray(bq, dtype=np.float32)
    bk = np.asarray(bk, dtype=np.float32)
    bv = np.asarray(bv, dtype=np.float32)
    bo = np.asarray(bo, dtype=np.float32)

    nc = _get_nc()

    xT = [np.ascontiguousarray(queries[b].T) for b in range(B)]
    kxT = [np.ascontiguousarray(keys[b].T) for b in range(B)]
    vxT = [np.ascontiguousarray(values[b].T) for b in range(B)]

    in_maps = []
    for c in range(NCORES):
        b, hg = c // 4, c % 4
        sl = slice(hg * HPC * DH, (hg + 1) * HPC * DH)
        in_maps.append({
            "xT": xT[b],
            "kxT": kxT[b],
            "vxT": vxT[b],
            "wq_s": np.ascontiguousarray(wq[:, sl]),
            "wk_s": np.ascontiguousarray(wk[:, sl]),
            "wv_s": np.ascontiguousarray(wv[:, sl]),
            "wo_s": np.ascontiguousarray(wo[sl, :]),
            "bq_s": np.ascontiguousarray(bq[sl].reshape(-1, 1)),
            "bk_s": np.ascontiguousarray(bk[sl].reshape(-1, 1)),
            "bv_s": np.ascontiguousarray(bv[sl].reshape(1, -1)),
        })

    results = _run_sharded(in_maps)

    out = np.empty((B, S, D), dtype=np.float32)
    weights = np.empty((B, H, S, S), dtype=np.float32)
    from concurrent.futures import ThreadPoolExecutor

    def fill_head(args):
        b, hg, h = args
        np.copyto(weights[b, hg * HPC + h], results[b * 4 + hg]["w_out"][h].T)

    with ThreadPoolExecutor(max_workers=16) as ex:
        list(ex.map(fill_head,
                    [(b, hg, h) for b in range(B) for hg in range(4)
                     for h in range(HPC)]))
    for b in range(B):
        acc = results[b * 4]["out_p"].copy()
        for hg in range(1, 4):
            acc += results[b * 4 + hg]["out_p"]
        out[b] = acc + bo
    return out, weights


# revision 44
# speedup vs baseline: 1.0002x; 1.0002x over previous
"""Multi-head attention (B=2, S=2048, D=1024, H=16) on 8 trn2 NeuronCores.

Sharding: core c = (batch b=c//4, head-group hg=c%4, heads hg*4..hg*4+3).
 - data-parallel over batch, tensor-parallel over heads (Megatron-style):
   wq/wk/wv column-sharded, wo row-sharded; partial out projections summed
   host-side at unshard time; bo added host-side.
Device kernel (per core, all matmuls in fp32r = TF32, fp32 accumulate):
 - q_T, k_T computed in transposed layout [head*64, token] so the QKV biases
   are per-partition; v computed in natural layout [token, head*64] augmented
   with a ones column per head (so the attention matmul's 65th output row is
   the softmax denominator Z for free).
 - scores s_T[k, q] per (head, q-chunk) on PE; exp on ACT straight out of
   PSUM (scale=1/8 folded); attn_T accumulated over k-tiles on PE.
 - normalization: recip(Z) on DVE, partition-broadcast via a K=1 PE outer
   product, applied by DVE. Weights written to DRAM as w_T[h, k, q] (2KB
   contiguous bursts); host transposes at unshard time.
"""
import sys
import os

sys.path.insert(0, "/opt/trn_rl_repo")

import numpy as np

B = 2
S = 2048
D = 1024
H = 16
DH = 64          # head dim
HPC = 4          # heads per core
NCORES = 8
P = 128          # partitions
QC = 512         # q-chunk (tokens per scores/attn matmul rhs)
NQC = S // QC    # 4
NKT = S // P     # 16 k-tiles
NDT = D // P     # 8 contraction tiles for projections

_cached = {}


def _build_nc():
    import concourse.tile as tile
    import concourse.mybir as mybir
    from concourse import bacc
    from concourse.bass import ds, ts

    f32 = mybir.dt.float32
    f32r = mybir.dt.float32r
    AF = mybir.ActivationFunctionType

    nc = bacc.Bacc("TRN2", target_bir_lowering=False, debug=False,
                   num_devices=NCORES)

    # ---- DRAM I/O ----
    xT = nc.dram_tensor("xT", [D, S], f32r, kind="ExternalInput").ap()
    kxT = nc.dram_tensor("kxT", [D, S], f32r, kind="ExternalInput").ap()
    vxT = nc.dram_tensor("vxT", [D, S], f32r, kind="ExternalInput").ap()
    wq_s = nc.dram_tensor("wq_s", [D, HPC * DH], f32r, kind="ExternalInput").ap()
    wk_s = nc.dram_tensor("wk_s", [D, HPC * DH], f32r, kind="ExternalInput").ap()
    wv_s = nc.dram_tensor("wv_s", [D, HPC * DH], f32r, kind="ExternalInput").ap()
    wo_s = nc.dram_tensor("wo_s", [HPC * DH, D], f32r, kind="ExternalInput").ap()
    bq_s = nc.dram_tensor("bq_s", [HPC * DH, 1], f32, kind="ExternalInput").ap()
    bk_s = nc.dram_tensor("bk_s", [HPC * DH, 1], f32, kind="ExternalInput").ap()
    bv_s = nc.dram_tensor("bv_s", [1, HPC * DH], f32, kind="ExternalInput").ap()

    w_out = nc.dram_tensor("w_out", [HPC, S, S], f32, kind="ExternalOutput").ap()
    out_p = nc.dram_tensor("out_p", [S, D], f32, kind="ExternalOutput").ap()

    with tile.TileContext(nc) as tc, \
         nc.allow_low_precision(reason="fp32r (tf32) matmul inputs are intentional"):
        import contextlib
        stack = contextlib.ExitStack()
        with stack:
            singles = stack.enter_context(tc.tile_pool(name="singles", bufs=1))
            wpool = stack.enter_context(tc.tile_pool(name="wpool", bufs=1))
            qkv = stack.enter_context(tc.tile_pool(name="qkv", bufs=1))
            # phase-A-only pools, released before attention pools allocate
            stack_a = stack.enter_context(contextlib.ExitStack())
            wproj = stack_a.enter_context(tc.tile_pool(name="wproj", bufs=1))
            xin = stack_a.enter_context(tc.tile_pool(name="xin", bufs=8))
            ps_proj = stack_a.enter_context(
                tc.tile_pool(name="ps_proj", bufs=8, space="PSUM"))

            # ---- weight slices & biases into SBUF ----
            # (wv first -- the v projection runs first; the rest of the
            # weights load behind the first input tensor's tiles)
            wq_sb = wproj.tile([P, NDT * 256], f32r)   # [128, (kt 8, col 256)]
            wk_sb = wproj.tile([P, NDT * 256], f32r)
            wv_sb = wproj.tile([P, NDT * 256], f32r)
            wo_sb = wpool.tile([P, 2 * D], f32r)       # [128, (pr 2, 1024)]
            nc.sync.dma_start(wv_sb[:].rearrange("p (kt c) -> p kt c", c=256),
                              wv_s.rearrange("(kt p) c -> p kt c", p=P))
            bv_bc = singles.tile([P, HPC * DH], f32)
            nc.sync.dma_start(bv_bc[:], bv_s[0, :].partition_broadcast(P))

            ones_col_f = singles.tile([P, 1], f32)     # fp32 ones source
            nc.vector.memset(ones_col_f[:], 1.0)
            ones_row_f = singles.tile([1, P], f32)
            nc.vector.memset(ones_row_f[:], 1.0)
            ones_row = singles.tile([1, P], f32r)      # lhsT for PE broadcast
            nc.vector.tensor_copy(ones_row[:], ones_row_f[:])
            ones_col = singles.tile([P, 1], f32r)      # lhsT for PE column sums
            nc.vector.tensor_copy(ones_col[:], ones_col_f[:])

            # persistent activations
            q_T = [qkv.tile([P, S], f32r, name=f"q_T{i}") for i in range(2)]
            k_T = [qkv.tile([P, S], f32r, name=f"k_T{i}") for i in range(2)]
            v_aug = [qkv.tile([P, HPC * (DH + 1)], f32r, name=f"v_aug{i}")
                     for i in range(NKT)]
            attn_all = [qkv.tile([P, S], f32r, name=f"attn_all{i}") for i in range(2)]

            # ---- phase A: projections ----
            aparts = os.environ.get("KERNEL_A_PARTS", "all")

            def proj_T(dst, w_sb, b_sb, src_dram):
                # kt-outer: each x-tile's matmuls run right after its load, so
                # the tile releases quickly and input DMA streams without
                # stalling on pool slots. 8 live PSUM accumulation groups.
                groups = {}
                for pr in range(2):
                    for tc4 in range(NQC):
                        groups[(pr, tc4)] = ps_proj.tile([P, QC], f32, tag="pp", name=f"ppg{pr}_{tc4}")
                for kt in range(NDT):
                    t = xin.tile([P, S], f32r, tag="xin")
                    nc.sync.dma_start(t[:], src_dram[ds(kt * P, P), :])
                    if aparts == "x":
                        continue
                    for pr in range(2):
                        for tc4 in range(NQC):
                            nc.tensor.matmul(
                                groups[(pr, tc4)][:],
                                w_sb[:, ds(kt * 256 + pr * P, P)],
                                t[:, ds(tc4 * QC, QC)],
                                start=(kt == 0), stop=(kt == NDT - 1))
                if aparts in ("x", "xm"):
                    return
                for pr in range(2):
                    for tc4 in range(NQC):
                        nc.vector.tensor_scalar_add(dst[pr][:, ds(tc4 * QC, QC)],
                                                    groups[(pr, tc4)][:],
                                                    b_sb[:, ds(pr, 1)])

            # v first (its consumers sit deepest in the attention chain),
            # then k, then q — scores fire right after q-proj completes.
            # v in natural layout, augmented ones column per head;
            # kt-outer over two passes of 8 token-chunks (8 PSUM banks each).
            skip_v = aparts in ("w", "x", "xm")
            vtiles = []
            for kt in (range(NDT) if not skip_v else []):
                t = xin.tile([P, S], f32r, tag="xin")
                nc.sync.dma_start(t[:], vxT[ds(kt * P, P), :])
                vtiles.append(t)
            for vpass in (range(2) if not skip_v else []):
                groups = [ps_proj.tile([P, HPC * DH], f32, tag="pp",
                                       name=f"ppv{vpass}_{gi}")
                          for gi in range(NKT // 2)]
                for kt in range(NDT):
                    for gi in range(NKT // 2):
                        tk = vpass * (NKT // 2) + gi
                        nc.tensor.matmul(
                            groups[gi][:],
                            vtiles[kt][:, ds(tk * P, P)],
                            wv_sb[:, ds(kt * 256, 256)],
                            start=(kt == 0), stop=(kt == NDT - 1))
                for gi in range(NKT // 2):
                    tk = vpass * (NKT // 2) + gi
                    va = v_aug[tk]
                    va3 = va[:].rearrange("p (h c) -> p h c", c=DH + 1)
                    nc.vector.tensor_add(
                        va3[:, :, 0:DH],
                        groups[gi][:].rearrange("p (h c) -> p h c", c=DH),
                        bv_bc[:].rearrange("p (h c) -> p h c", c=DH))
                    nc.vector.tensor_copy(
                        va3[:, :, DH:DH + 1],
                        ones_col_f[:, None, 0:1].broadcast_to([P, HPC, 1]))

            nc.sync.dma_start(wk_sb[:].rearrange("p (kt c) -> p kt c", c=256),
                              wk_s.rearrange("(kt p) c -> p kt c", p=P))
            nc.sync.dma_start(wq_sb[:].rearrange("p (kt c) -> p kt c", c=256),
                              wq_s.rearrange("(kt p) c -> p kt c", p=P))
            nc.sync.dma_start(wo_sb[:].rearrange("p (pr c) -> p pr c", c=D),
                              wo_s.rearrange("(pr p) c -> p pr c", p=P))
            bq_sb = singles.tile([P, 2], f32)
            bk_sb = singles.tile([P, 2], f32)
            nc.sync.dma_start(bq_sb[:].rearrange("p (pr o) -> p pr o", o=1),
                              bq_s.rearrange("(pr p) o -> p pr o", p=P))
            nc.sync.dma_start(bk_sb[:].rearrange("p (pr o) -> p pr o", o=1),
                              bk_s.rearrange("(pr p) o -> p pr o", p=P))

            if aparts != "w":
                proj_T(k_T, wk_sb, bk_sb, kxT)
                proj_T(q_T, wq_sb, bq_sb, xT)

            # ---- release phase-A pools, allocate attention pools ----
            stack_a.close()
            ps_scores = stack.enter_context(
                tc.tile_pool(name="ps_scores", bufs=3, space="PSUM"))
            ps_attn = stack.enter_context(
                tc.tile_pool(name="ps_attn", bufs=2, space="PSUM"))
            ps_bc = stack.enter_context(
                tc.tile_pool(name="ps_bc", bufs=1, space="PSUM"))
            ps_out = stack.enter_context(
                tc.tile_pool(name="ps_out", bufs=2, space="PSUM"))
            expp = stack.enter_context(tc.tile_pool(name="expp", bufs=6))
            woutp = stack.enter_context(tc.tile_pool(name="woutp", bufs=4))
            small = stack.enter_context(tc.tile_pool(name="small", bufs=4))
            outp = stack.enter_context(tc.tile_pool(name="outp", bufs=3))

            # ---- phase B: attention per (head, q-chunk) ----
            phases = os.environ.get("KERNEL_PHASES", "abc")
            if skip_v:
                phases = ""
            for qc in (range(NQC) if "b" in phases else []):
                for h in range(HPC):
                    pr = h // 2
                    off = (h % 2) * DH
                    ecs = []
                    for ktg in range(4):
                        ec = expp.tile([P, 4 * QC], f32r, tag="ec")
                        for j in range(4):
                            kt = ktg * 4 + j
                            pss = ps_scores.tile([P, QC], f32, tag="ps")
                            nc.tensor.matmul(
                                pss[:],
                                k_T[pr][off:off + DH, ds(kt * P, P)],
                                q_T[pr][off:off + DH, ds(qc * QC, QC)],
                                start=True, stop=True)
                            nc.scalar.activation(ec[:, ds(j * QC, QC)], pss[:],
                                                 AF.Exp, scale=0.125)
                        ecs.append(ec)
                    bparts = os.environ.get("KERNEL_B_PARTS", "all")
                    if bparts == "se":
                        continue
                    # attn_T (+Z row) accumulated over all 16 k-tiles
                    pa = ps_attn.tile([DH + 1, QC], f32, tag="pa")
                    for ktg in range(4):
                        for j in range(4):
                            kt = ktg * 4 + j
                            nc.tensor.matmul(
                                pa[:],
                                v_aug[kt][:, ds(h * (DH + 1), DH + 1)],
                                ecs[ktg][:, ds(j * QC, QC)],
                                start=(kt == 0), stop=(kt == NKT - 1))
                    # recip(Z) and partition-broadcast via K=1 outer product
                    tz = small.tile([1, QC], f32r, tag="tz")
                    nc.vector.reciprocal(tz[:], pa[DH:DH + 1, :])
                    pb = ps_bc.tile([P, QC], f32, tag="pb")
                    nc.tensor.matmul(pb[:], ones_row[:], tz[:], start=True, stop=True)
                    # SBUF copy of the broadcast factor (GpSimd can't read PSUM)
                    pb_s = small.tile([P, QC], f32, tag="pb_s")
                    nc.vector.tensor_copy(pb_s[:], pb[:])
                    # normalize attn rows into attn_all
                    nc.vector.tensor_mul(
                        attn_all[pr][off:off + DH, ds(qc * QC, QC)],
                        pa[0:DH, :], pb_s[0:DH, :])
                    if bparts == "sea":
                        continue
                    # normalize weights into fp32 staging tiles, DMA out
                    # (split DVE / GpSimd to balance engine load)
                    for ktg in range(4):
                        ec = ecs[ktg]
                        ecv = ec[:].rearrange("p (j q) -> p j q", j=4)
                        wn = woutp.tile([P, 4 * QC], f32, tag="wn")
                        wnv = wn[:].rearrange("p (j q) -> p j q", j=4)
                        if ktg % 2 == 0:
                            nc.vector.tensor_mul(
                                wnv, ecv,
                                pb[:, None, :].broadcast_to([P, 4, QC]))
                        else:
                            nc.gpsimd.tensor_mul(
                                wnv, ecv,
                                pb_s[:, None, :].broadcast_to([P, 4, QC]))
                        nc.sync.dma_start(
                            w_out[h, ds(ktg * 4 * P, 4 * P), ds(qc * QC, QC)]
                            .rearrange("(j p) q -> p j q", p=P),
                            wnv)

                # ---- output projection for this q-chunk (all 4 heads done) ----
                if "c" not in phases:
                    continue
                for sub in range(NQC):
                    tk = qc * NQC + sub
                    for oc in range(2):
                        po = ps_out.tile([P, QC], f32, tag="po")
                        for pr in range(2):
                            nc.tensor.matmul(
                                po[:],
                                attn_all[pr][:, ds(tk * P, P)],
                                wo_sb[:, ds(pr * D + oc * QC, QC)],
                                start=(pr == 0), stop=(pr == 1))
                        to = outp.tile([P, QC], f32, tag="to")
                        nc.vector.tensor_copy(to[:], po[:])
                        nc.sync.dma_start(
                            out_p[tk * P:(tk + 1) * P, ds(oc * QC, QC)], to[:])

    nc.compile()
    return nc


def _get_nc():
    if "nc" not in _cached:
        _cached["nc"] = _build_nc()
    return _cached["nc"]


def _get_sharded():
    """Build (once) a jitted shard_map callable over the bass module,
    mirroring concourse.bass2jax.run_bass_via_pjrt but cached so repeat
    calls skip retracing, and usable for device-time benchmarking."""
    if "sharded" in _cached:
        return _cached["sharded"]
    import jax
    import jax.numpy as jnp
    from jax.sharding import Mesh, PartitionSpec
    from jax.experimental.shard_map import shard_map
    import concourse.mybir as mybir
    from concourse.bass2jax import (_bass_exec_p, install_neuronx_cc_hook,
                                    partition_id_tensor)

    nc = _get_nc()
    install_neuronx_cc_hook()

    partition_name = (nc.partition_id_tensor.name
                      if nc.partition_id_tensor else None)
    in_names, out_names, out_avals, zero_shapes = [], [], [], []
    for alloc in nc.m.functions[0].allocations:
        if not isinstance(alloc, mybir.MemoryLocationSet):
            continue
        name = alloc.memorylocations[0].name
        if alloc.kind == "ExternalInput":
            if name == partition_name:
                continue
            in_names.append(name)
        elif alloc.kind == "ExternalOutput":
            shape = tuple(alloc.tensor_shape)
            dtype = mybir.dt.np(alloc.dtype)
            out_names.append(name)
            out_avals.append(jax.core.ShapedArray(shape, dtype))
            zero_shapes.append((shape, dtype))
    n_params = len(in_names)
    all_names = in_names + out_names
    if partition_name is not None:
        all_names = all_names + [partition_name]
    donate = tuple(range(n_params, n_params + len(out_names)))

    def _body(*args):
        operands = list(args)
        if partition_name is not None:
            operands.append(partition_id_tensor())
        outs = _bass_exec_p.bind(
            *operands,
            out_avals=tuple(out_avals),
            in_names=tuple(all_names),
            out_names=tuple(out_names),
            lowering_input_output_aliases=(),
            sim_require_finite=True,
            sim_require_nnan=True,
            nc=nc,
        )
        return tuple(outs)

    devices = jax.devices()[:NCORES]
    mesh = Mesh(np.asarray(devices), ("core",))
    nio = n_params + len(out_names)
    sharded = jax.jit(
        shard_map(_body, mesh=mesh,
                  in_specs=(PartitionSpec("core"),) * nio,
                  out_specs=(PartitionSpec("core"),) * len(out_names),
                  check_rep=False),
        donate_argnums=donate, keep_unused=True)

    out_sh = jax.sharding.NamedSharding(mesh, PartitionSpec("core"))

    zero_fns = [
        jax.jit(lambda s=shape, d=dtype: jnp.zeros((NCORES * s[0], *s[1:]), d),
                out_shardings=out_sh)
        for shape, dtype in zero_shapes
    ]

    def make_zeros():
        return [f() for f in zero_fns]

    _cached["sharded"] = dict(
        fn=sharded, in_names=in_names, out_names=out_names,
        out_avals=out_avals, make_zeros=make_zeros, mesh=mesh, sharding=out_sh)
    return _cached["sharded"]


def _run_sharded(in_maps):
    import jax
    sh = _get_sharded()
    concat_in = [
        np.concatenate([np.asarray(in_maps[c][name]) for c in range(NCORES)], axis=0)
        for name in sh["in_names"]
    ]
    dev_in = [jax.device_put(a, sh["sharding"]) for a in concat_in]
    zeros = sh["make_zeros"]()
    out_arrs = sh["fn"](*dev_in, *zeros)
    _cached["last_dev_in"] = dev_in
    gathered = [
        np.asarray(a).reshape(NCORES, *sh["out_avals"][i].shape)
        for i, a in enumerate(out_arrs)
    ]
    return [
        {name: gathered[i][c] for i, name in enumerate(sh["out_names"])}
        for c in range(NCORES)
    ]


def bench(n_iters=5):
    """Re-execute the compiled NEFF with device-resident inputs; returns list
    of per-iteration wall seconds (dispatch + HW exec, no host transfers)."""
    import time
    import jax
    sh = _get_sharded()
    dev_in = _cached["last_dev_in"]
    times = []
    for _ in range(n_iters):
        zeros = sh["make_zeros"]()
        for z in zeros:
            z.block_until_ready()
        t0 = time.perf_counter()
        outs = sh["fn"](*dev_in, *zeros)
        for o in outs:
            o.block_until_ready()
        times.append(time.perf_counter() - t0)
    return times


def kernel(queries, keys, values, wq, bq, wk, bk, wv, bv, wo, bo):
    queries = np.ascontiguousarray(np.asarray(queries, dtype=np.float32))
    keys = np.ascontiguousarray(np.asarray(keys, dtype=np.float32))
    values = np.ascontiguousarray(np.asarray(values, dtype=np.float32))
    wq = np.asarray(wq, dtype=np.float32)
    wk = np.asarray(wk, dtype=np.float32)
    wv = np.asarray(wv, dtype=np.float32)
    wo = np.asarray(wo, dtype=np.float32)
    bq = np.asarray(bq, dtype=np.float32)
    bk = np.asarray(bk, dtype=np.float32)
    bv = np.asarray(bv, dtype=np.float32)
    bo = np.asarray(bo, dtype=np.float32)

    nc = _get_nc()

    xT = [np.ascontiguousarray(queries[b].T) for b in range(B)]
    kxT = [np.ascontiguousarray(keys[b].T) for b in range(B)]
    vxT = [np.ascontiguousarray(values[b].T) for b in range(B)]

    in_maps = []
    for c in range(NCORES):
        b, hg = c // 4, c % 4
        sl = slice(hg * HPC * DH, (hg + 1) * HPC * DH)
        in_maps.append({
            "xT": xT[b],
            "kxT": kxT[b],
            "vxT": vxT[b],
            "wq_s": np.ascontiguousarray(wq[:, sl]),
            "wk_s": np.ascontiguousarray(wk[:, sl]),
            "wv_s": np.ascontiguousarray(wv[:, sl]),
            "wo_s": np.ascontiguousarray(wo[sl, :]),
            "bq_s": np.ascontiguousarray(bq[sl].reshape(-1, 1)),
            "bk_s": np.ascontiguousarray(bk[sl].reshape(-1, 1)),
            "bv_s": np.ascontiguousarray(bv[sl].reshape(1, -1)),
        })

    results = _run_sharded(in_maps)

    out = np.empty((B, S, D), dtype=np.float32)
    weights = np.empty((B, H, S, S), dtype=np.float32)
    from concurrent.futures import ThreadPoolExecutor

    def fill_head(args):
        b, hg, h = args
        np.copyto(weights[b, hg * HPC + h], results[b * 4 + hg]["w_out"][h].T)

    with ThreadPoolExecutor(max_workers=16) as ex:
        list(ex.map(fill_head,
                    [(b, hg, h) for b in range(B) for hg in range(4)
                     for h in range(HPC)]))
    for b in range(B):
        acc = results[b * 4]["out_p"].copy()
        for hg in range(1, 4):
            acc += results[b * 4 + hg]["out_p"]
        out[b] = acc + bo
    return out, weights


# revision 45
# speedup vs baseline: 1.2651x; 1.2649x over previous
"""Multi-head attention (B=2, S=2048, D=1024, H=16) on 8 trn2 NeuronCores.

Sharding: core c = (batch b=c//4, head-group hg=c%4, heads hg*4..hg*4+3).
 - data-parallel over batch, tensor-parallel over heads (Megatron-style):
   wq/wk/wv column-sharded, wo row-sharded; partial out projections summed
   host-side at unshard time; bo added host-side.
Device kernel (per core, all matmuls in fp32r = TF32, fp32 accumulate):
 - q_T, k_T computed in transposed layout [head*64, token] so the QKV biases
   are per-partition; v computed in natural layout [token, head*64] augmented
   with a ones column per head (so the attention matmul's 65th output row is
   the softmax denominator Z for free).
 - scores s_T[k, q] per (head, q-chunk) on PE; exp on ACT straight out of
   PSUM (scale=1/8 folded); attn_T accumulated over k-tiles on PE.
 - normalization: recip(Z) on DVE, partition-broadcast via a K=1 PE outer
   product, applied by DVE. Weights written to DRAM as w_T[h, k, q] (2KB
   contiguous bursts); host transposes at unshard time.
"""
import sys
import os

sys.path.insert(0, "/opt/trn_rl_repo")

import numpy as np

B = 2
S = 2048
D = 1024
H = 16
DH = 64          # head dim
HPC = 4          # heads per core
NCORES = 8
P = 128          # partitions
QC = 512         # q-chunk (tokens per scores/attn matmul rhs)
NQC = S // QC    # 4
NKT = S // P     # 16 k-tiles
NDT = D // P     # 8 contraction tiles for projections

_cached = {}


def _build_nc():
    import concourse.tile as tile
    import concourse.mybir as mybir
    from concourse import bacc
    from concourse.bass import ds, ts

    f32 = mybir.dt.float32
    f32r = mybir.dt.float32r
    AF = mybir.ActivationFunctionType

    nc = bacc.Bacc("TRN2", target_bir_lowering=False, debug=False,
                   num_devices=NCORES)

    # ---- DRAM I/O ----
    xT = nc.dram_tensor("xT", [D, S], f32r, kind="ExternalInput").ap()
    kxT = nc.dram_tensor("kxT", [D, S], f32r, kind="ExternalInput").ap()
    vxT = nc.dram_tensor("vxT", [D, S], f32r, kind="ExternalInput").ap()
    wq_s = nc.dram_tensor("wq_s", [D, HPC * DH], f32r, kind="ExternalInput").ap()
    wk_s = nc.dram_tensor("wk_s", [D, HPC * DH], f32r, kind="ExternalInput").ap()
    wv_s = nc.dram_tensor("wv_s", [D, HPC * DH], f32r, kind="ExternalInput").ap()
    wo_s = nc.dram_tensor("wo_s", [HPC * DH, D], f32r, kind="ExternalInput").ap()
    bq_s = nc.dram_tensor("bq_s", [HPC * DH, 1], f32, kind="ExternalInput").ap()
    bk_s = nc.dram_tensor("bk_s", [HPC * DH, 1], f32, kind="ExternalInput").ap()
    bv_s = nc.dram_tensor("bv_s", [1, HPC * DH], f32, kind="ExternalInput").ap()

    w_out = nc.dram_tensor("w_out", [HPC, S, S], f32, kind="ExternalOutput").ap()
    out_p = nc.dram_tensor("out_p", [S, D], f32, kind="ExternalOutput").ap()

    with tile.TileContext(nc) as tc, \
         nc.allow_low_precision(reason="fp32r (tf32) matmul inputs are intentional"):
        import contextlib
        stack = contextlib.ExitStack()
        with stack:
            singles = stack.enter_context(tc.tile_pool(name="singles", bufs=1))
            wpool = stack.enter_context(tc.tile_pool(name="wpool", bufs=1))
            qkv = stack.enter_context(tc.tile_pool(name="qkv", bufs=1))
            # phase-A-only pools, released before attention pools allocate
            stack_a = stack.enter_context(contextlib.ExitStack())
            wproj = stack_a.enter_context(tc.tile_pool(name="wproj", bufs=1))
            xin = stack_a.enter_context(tc.tile_pool(name="xin", bufs=8))
            ps_proj = stack_a.enter_context(
                tc.tile_pool(name="ps_proj", bufs=8, space="PSUM"))

            # ---- weight slices & biases into SBUF ----
            # (wv first -- the v projection runs first; the rest of the
            # weights load behind the first input tensor's tiles)
            wq_sb = wproj.tile([P, NDT * 256], f32r)   # [128, (kt 8, col 256)]
            wk_sb = wproj.tile([P, NDT * 256], f32r)
            wv_sb = wproj.tile([P, NDT * 256], f32r)
            wo_sb = wpool.tile([P, 2 * D], f32r)       # [128, (pr 2, 1024)]
            nc.sync.dma_start(wv_sb[:].rearrange("p (kt c) -> p kt c", c=256),
                              wv_s.rearrange("(kt p) c -> p kt c", p=P))
            bv_bc = singles.tile([P, HPC * DH], f32)
            nc.sync.dma_start(bv_bc[:], bv_s[0, :].partition_broadcast(P))

            ones_col_f = singles.tile([P, 1], f32)     # fp32 ones source
            nc.vector.memset(ones_col_f[:], 1.0)
            ones_row_f = singles.tile([1, P], f32)
            nc.vector.memset(ones_row_f[:], 1.0)
            ones_row = singles.tile([1, P], f32r)      # lhsT for PE broadcast
            nc.vector.tensor_copy(ones_row[:], ones_row_f[:])
            ones_col = singles.tile([P, 1], f32r)      # lhsT for PE column sums
            nc.vector.tensor_copy(ones_col[:], ones_col_f[:])

            # persistent activations
            q_T = [qkv.tile([P, S], f32r, name=f"q_T{i}") for i in range(2)]
            k_T = [qkv.tile([P, S], f32r, name=f"k_T{i}") for i in range(2)]
            v_aug = [qkv.tile([P, HPC * (DH + 1)], f32r, name=f"v_aug{i}")
                     for i in range(NKT)]
            attn_all = [qkv.tile([P, S], f32r, name=f"attn_all{i}") for i in range(2)]

            # ---- phase A: projections ----
            aparts = os.environ.get("KERNEL_A_PARTS", "all")

            def proj_T(dst, w_sb, b_sb, src_dram):
                # kt-outer: each x-tile's matmuls run right after its load, so
                # the tile releases quickly and input DMA streams without
                # stalling on pool slots. 8 live PSUM accumulation groups.
                groups = {}
                for pr in range(2):
                    for tc4 in range(NQC):
                        groups[(pr, tc4)] = ps_proj.tile([P, QC], f32, tag="pp", name=f"ppg{pr}_{tc4}")
                for kt in range(NDT):
                    t = xin.tile([P, S], f32r, tag="xin")
                    nc.sync.dma_start(t[:], src_dram[ds(kt * P, P), :])
                    if aparts == "x":
                        continue
                    for pr in range(2):
                        for tc4 in range(NQC):
                            nc.tensor.matmul(
                                groups[(pr, tc4)][:],
                                w_sb[:, ds(kt * 256 + pr * P, P)],
                                t[:, ds(tc4 * QC, QC)],
                                start=(kt == 0), stop=(kt == NDT - 1))
                if aparts in ("x", "xm"):
                    return
                for pr in range(2):
                    for tc4 in range(NQC):
                        nc.vector.tensor_scalar_add(dst[pr][:, ds(tc4 * QC, QC)],
                                                    groups[(pr, tc4)][:],
                                                    b_sb[:, ds(pr, 1)])

            # v first (its consumers sit deepest in the attention chain),
            # then k, then q — scores fire right after q-proj completes.
            # v in natural layout, augmented ones column per head;
            # kt-outer over two passes of 8 token-chunks (8 PSUM banks each).
            skip_v = aparts in ("w", "x", "xm")
            vtiles = []
            for kt in (range(NDT) if not skip_v else []):
                t = xin.tile([P, S], f32r, tag="xin")
                nc.sync.dma_start(t[:], vxT[ds(kt * P, P), :])
                vtiles.append(t)
            for vpass in (range(2) if not skip_v else []):
                groups = [ps_proj.tile([P, HPC * DH], f32, tag="pp",
                                       name=f"ppv{vpass}_{gi}")
                          for gi in range(NKT // 2)]
                for kt in range(NDT):
                    for gi in range(NKT // 2):
                        tk = vpass * (NKT // 2) + gi
                        nc.tensor.matmul(
                            groups[gi][:],
                            vtiles[kt][:, ds(tk * P, P)],
                            wv_sb[:, ds(kt * 256, 256)],
                            start=(kt == 0), stop=(kt == NDT - 1))
                for gi in range(NKT // 2):
                    tk = vpass * (NKT // 2) + gi
                    va = v_aug[tk]
                    va3 = va[:].rearrange("p (h c) -> p h c", c=DH + 1)
                    nc.vector.tensor_add(
                        va3[:, :, 0:DH],
                        groups[gi][:].rearrange("p (h c) -> p h c", c=DH),
                        bv_bc[:].rearrange("p (h c) -> p h c", c=DH))
                    nc.vector.tensor_copy(
                        va3[:, :, DH:DH + 1],
                        ones_col_f[:, None, 0:1].broadcast_to([P, HPC, 1]))

            nc.sync.dma_start(wk_sb[:].rearrange("p (kt c) -> p kt c", c=256),
                              wk_s.rearrange("(kt p) c -> p kt c", p=P))
            nc.sync.dma_start(wq_sb[:].rearrange("p (kt c) -> p kt c", c=256),
                              wq_s.rearrange("(kt p) c -> p kt c", p=P))
            nc.sync.dma_start(wo_sb[:].rearrange("p (pr c) -> p pr c", c=D),
                              wo_s.rearrange("(pr p) c -> p pr c", p=P))
            bq_sb = singles.tile([P, 2], f32)
            bk_sb = singles.tile([P, 2], f32)
            nc.sync.dma_start(bq_sb[:].rearrange("p (pr o) -> p pr o", o=1),
                              bq_s.rearrange("(pr p) o -> p pr o", p=P))
            nc.sync.dma_start(bk_sb[:].rearrange("p (pr o) -> p pr o", o=1),
                              bk_s.rearrange("(pr p) o -> p pr o", p=P))

            if aparts != "w":
                proj_T(k_T, wk_sb, bk_sb, kxT)
                proj_T(q_T, wq_sb, bq_sb, xT)

            # ---- release phase-A pools, allocate attention pools ----
            stack_a.close()
            ps_scores = stack.enter_context(
                tc.tile_pool(name="ps_scores", bufs=3, space="PSUM"))
            ps_attn = stack.enter_context(
                tc.tile_pool(name="ps_attn", bufs=2, space="PSUM"))
            ps_bc = stack.enter_context(
                tc.tile_pool(name="ps_bc", bufs=1, space="PSUM"))
            ps_out = stack.enter_context(
                tc.tile_pool(name="ps_out", bufs=2, space="PSUM"))
            expp = stack.enter_context(tc.tile_pool(name="expp", bufs=6))
            woutp = stack.enter_context(tc.tile_pool(name="woutp", bufs=4))
            small = stack.enter_context(tc.tile_pool(name="small", bufs=4))
            outp = stack.enter_context(tc.tile_pool(name="outp", bufs=3))

            # ---- phase B: attention per (head, q-chunk) ----
            phases = os.environ.get("KERNEL_PHASES", "abc")
            if skip_v:
                phases = ""
            for qc in (range(NQC) if "b" in phases else []):
                for h in range(HPC):
                    pr = h // 2
                    off = (h % 2) * DH
                    ecs = []
                    for ktg in range(4):
                        ec = expp.tile([P, 4 * QC], f32r, tag="ec")
                        for j in range(4):
                            kt = ktg * 4 + j
                            pss = ps_scores.tile([P, QC], f32, tag="ps")
                            nc.tensor.matmul(
                                pss[:],
                                k_T[pr][off:off + DH, ds(kt * P, P)],
                                q_T[pr][off:off + DH, ds(qc * QC, QC)],
                                start=True, stop=True)
                            nc.scalar.activation(ec[:, ds(j * QC, QC)], pss[:],
                                                 AF.Exp, scale=0.125)
                        ecs.append(ec)
                    bparts = os.environ.get("KERNEL_B_PARTS", "all")
                    if bparts == "se":
                        continue
                    # attn_T (+Z row) accumulated over all 16 k-tiles
                    pa = ps_attn.tile([DH + 1, QC], f32, tag="pa")
                    for ktg in range(4):
                        for j in range(4):
                            kt = ktg * 4 + j
                            nc.tensor.matmul(
                                pa[:],
                                v_aug[kt][:, ds(h * (DH + 1), DH + 1)],
                                ecs[ktg][:, ds(j * QC, QC)],
                                start=(kt == 0), stop=(kt == NKT - 1))
                    # recip(Z) and partition-broadcast via K=1 outer product
                    tz = small.tile([1, QC], f32r, tag="tz")
                    nc.vector.reciprocal(tz[:], pa[DH:DH + 1, :])
                    pb = ps_bc.tile([P, QC], f32, tag="pb")
                    nc.tensor.matmul(pb[:], ones_row[:], tz[:], start=True, stop=True)
                    # SBUF copy of the broadcast factor (GpSimd can't read PSUM)
                    pb_s = small.tile([P, QC], f32, tag="pb_s")
                    nc.vector.tensor_copy(pb_s[:], pb[:])
                    # normalize attn rows into attn_all
                    nc.vector.tensor_mul(
                        attn_all[pr][off:off + DH, ds(qc * QC, QC)],
                        pa[0:DH, :], pb_s[0:DH, :])
                    if bparts == "sea":
                        continue
                    # normalize weights into fp32 staging tiles, DMA out
                    # (split DVE / GpSimd to balance engine load)
                    for ktg in range(4):
                        ec = ecs[ktg]
                        ecv = ec[:].rearrange("p (j q) -> p j q", j=4)
                        wn = woutp.tile([P, 4 * QC], f32, tag="wn")
                        wnv = wn[:].rearrange("p (j q) -> p j q", j=4)
                        if ktg % 2 == 0:
                            nc.vector.tensor_mul(
                                wnv, ecv,
                                pb[:, None, :].broadcast_to([P, 4, QC]))
                        else:
                            nc.gpsimd.tensor_mul(
                                wnv, ecv,
                                pb_s[:, None, :].broadcast_to([P, 4, QC]))
                        nc.sync.dma_start(
                            w_out[h, ds(ktg * 4 * P, 4 * P), ds(qc * QC, QC)]
                            .rearrange("(j p) q -> p j q", p=P),
                            wnv)

                # ---- output projection for this q-chunk (all 4 heads done) ----
                if "c" not in phases:
                    continue
                for sub in range(NQC):
                    tk = qc * NQC + sub
                    for oc in range(2):
                        po = ps_out.tile([P, QC], f32, tag="po")
                        for pr in range(2):
                            nc.tensor.matmul(
                                po[:],
                                attn_all[pr][:, ds(tk * P, P)],
                                wo_sb[:, ds(pr * D + oc * QC, QC)],
                                start=(pr == 0), stop=(pr == 1))
                        to = outp.tile([P, QC], f32, tag="to")
                        nc.vector.tensor_copy(to[:], po[:])
                        nc.sync.dma_start(
                            out_p[tk * P:(tk + 1) * P, ds(oc * QC, QC)], to[:])

    nc.compile()
    return nc


def _get_nc():
    if "nc" not in _cached:
        _cached["nc"] = _build_nc()
    return _cached["nc"]


def _get_sharded():
    """Build (once) a jitted shard_map callable over the bass module,
    mirroring concourse.bass2jax.run_bass_via_pjrt but cached so repeat
    calls skip retracing, and usable for device-time benchmarking."""
    if "sharded" in _cached:
        return _cached["sharded"]
    import jax
    import jax.numpy as jnp
    from jax.sharding import Mesh, PartitionSpec
    from jax.experimental.shard_map import shard_map
    import concourse.mybir as mybir
    from concourse.bass2jax import (_bass_exec_p, install_neuronx_cc_hook,
                                    partition_id_tensor)

    nc = _get_nc()
    install_neuronx_cc_hook()

    partition_name = (nc.partition_id_tensor.name
                      if nc.partition_id_tensor else None)
    in_names, out_names, out_avals, zero_shapes = [], [], [], []
    for alloc in nc.m.functions[0].allocations:
        if not isinstance(alloc, mybir.MemoryLocationSet):
            continue
        name = alloc.memorylocations[0].name
        if alloc.kind == "ExternalInput":
            if name == partition_name:
                continue
            in_names.append(name)
        elif alloc.kind == "ExternalOutput":
            shape = tuple(alloc.tensor_shape)
            dtype = mybir.dt.np(alloc.dtype)
            out_names.append(name)
            out_avals.append(jax.core.ShapedArray(shape, dtype))
            zero_shapes.append((shape, dtype))
    n_params = len(in_names)
    all_names = in_names + out_names
    if partition_name is not None:
        all_names = all_names + [partition_name]
    donate = tuple(range(n_params, n_params + len(out_names)))

    def _body(*args):
        operands = list(args)
        if partition_name is not None:
            operands.append(partition_id_tensor())
        outs = _bass_exec_p.bind(
            *operands,
            out_avals=tuple(out_avals),
            in_names=tuple(all_names),
            out_names=tuple(out_names),
            lowering_input_output_aliases=(),
            sim_require_finite=True,
            sim_require_nnan=True,
            nc=nc,
        )
        return tuple(outs)

    devices = jax.devices()[:NCORES]
    mesh = Mesh(np.asarray(devices), ("core",))
    nio = n_params + len(out_names)
    sharded = jax.jit(
        shard_map(_body, mesh=mesh,
                  in_specs=(PartitionSpec("core"),) * nio,
                  out_specs=(PartitionSpec("core"),) * len(out_names),
                  check_rep=False),
        donate_argnums=donate, keep_unused=True)

    out_sh = jax.sharding.NamedSharding(mesh, PartitionSpec("core"))

    zero_fns = [
        jax.jit(lambda s=shape, d=dtype: jnp.zeros((NCORES * s[0], *s[1:]), d),
                out_shardings=out_sh)
        for shape, dtype in zero_shapes
    ]

    def make_zeros():
        return [f() for f in zero_fns]

    _cached["sharded"] = dict(
        fn=sharded, in_names=in_names, out_names=out_names,
        out_avals=out_avals, make_zeros=make_zeros, mesh=mesh, sharding=out_sh)
    return _cached["sharded"]


def _run_sharded(in_maps):
    import jax
    sh = _get_sharded()
    concat_in = [
        np.concatenate([np.asarray(in_maps[c][name]) for c in range(NCORES)], axis=0)
        for name in sh["in_names"]
    ]
    dev_in = [jax.device_put(a, sh["sharding"]) for a in concat_in]
    zeros = sh["make_zeros"]()
    out_arrs = sh["fn"](*dev_in, *zeros)
    _cached["last_dev_in"] = dev_in
    gathered = [
        np.asarray(a).reshape(NCORES, *sh["out_avals"][i].shape)
        for i, a in enumerate(out_arrs)
    ]
    return [
        {name: gathered[i][c] for i, name in enumerate(sh["out_names"])}
        for c in range(NCORES)
    ]


def bench(n_iters=5):
    """Re-execute the compiled NEFF with device-resident inputs; returns list
    of per-iteration wall seconds (dispatch + HW exec, no host transfers)."""
    import time
    import jax
    sh = _get_sharded()
    dev_in = _cached["last_dev_in"]
    times = []
    for _ in range(n_iters):
        zeros = sh["make_zeros"]()
        for z in zeros:
            z.block_until_ready()
        t0 = time.perf_counter()
        outs = sh["fn"](*dev_in, *zeros)
        for o in outs:
            o.block_until_ready()
        times.append(time.perf_counter() - t0)
    return times


def kernel(queries, keys, values, wq, bq, wk, bk, wv, bv, wo, bo):
    queries = np.ascontiguousarray(np.asarray(queries, dtype=np.float32))
    keys = np.ascontiguousarray(np.asarray(keys, dtype=np.float32))
    values = np.ascontiguousarray(np.asarray(values, dtype=np.float32))
    wq = np.asarray(wq, dtype=np.float32)
    wk = np.asarray(wk, dtype=np.float32)
    wv = np.asarray(wv, dtype=np.float32)
    wo = np.asarray(wo, dtype=np.float32)
    bq = np.asarray(bq, dtype=np.float32)
    bk = np.asarray(bk, dtype=np.float32)
    bv = np.asarray(bv, dtype=np.float32)
    bo = np.asarray(bo, dtype=np.float32)

    nc = _get_nc()

    xT = [np.ascontiguousarray(queries[b].T) for b in range(B)]
    kxT = [np.ascontiguousarray(keys[b].T) for b in range(B)]
    vxT = [np.ascontiguousarray(values[b].T) for b in range(B)]

    in_maps = []
    for c in range(NCORES):
        b, hg = c // 4, c % 4
        sl = slice(hg * HPC * DH, (hg + 1) * HPC * DH)
        in_maps.append({
            "xT": xT[b],
            "kxT": kxT[b],
            "vxT": vxT[b],
            "wq_s": np.ascontiguousarray(wq[:, sl]),
            "wk_s": np.ascontiguousarray(wk[:, sl]),
            "wv_s": np.ascontiguousarray(wv[:, sl]),
            "wo_s": np.ascontiguousarray(wo[sl, :]),
            "bq_s": np.ascontiguousarray(bq[sl].reshape(-1, 1)),
            "bk_s": np.ascontiguousarray(bk[sl].reshape(-1, 1)),
            "bv_s": np.ascontiguousarray(bv[sl].reshape(1, -1)),
        })

    import jax
    from concurrent.futures import ThreadPoolExecutor

    sh = _get_sharded()
    concat_in = [
        np.concatenate([np.asarray(in_maps[c][name]) for c in range(NCORES)],
                       axis=0)
        for name in sh["in_names"]
    ]
    dev_in = [jax.device_put(a, sh["sharding"]) for a in concat_in]
    zeros = sh["make_zeros"]()
    out_arrs = sh["fn"](*dev_in, *zeros)
    _cached["last_dev_in"] = dev_in
    arr_by_name = dict(zip(sh["out_names"], out_arrs))

    out = np.empty((B, S, D), dtype=np.float32)
    weights = np.empty((B, H, S, S), dtype=np.float32)

    # Pipeline: fetch w_out shards (serial tunnel) while worker threads
    # transpose already-fetched shards into the weights output.
    w_shards = sorted(arr_by_name["w_out"].addressable_shards,
                      key=lambda s: s.index[0].start or 0)
    with ThreadPoolExecutor(max_workers=8) as ex:
        futs = []

        def fill(c, data):
            b, hg = c // 4, c % 4
            for h in range(HPC):
                np.copyto(weights[b, hg * HPC + h], data[h].T)

        for c, s in enumerate(w_shards):
            data = np.asarray(s.data)
            futs.append(ex.submit(fill, c, data))
        op = np.asarray(arr_by_name["out_p"]).reshape(NCORES, S, D)
        for f in futs:
            f.result()

    for b in range(B):
        acc = op[b * 4].copy()
        for hg in range(1, 4):
            acc += op[b * 4 + hg]
        out[b] = acc + bo
    return out, weights
